# revision 1
# baseline (speedup 1.0000x reference)
"""Trainium2 Bass kernel for the sparse CG tensor product (CudaTensorProduct).

Math: out[b, o] = sum_k cb[k] * in1[b, i1[k]] * in2[b, i2[k]]  (scatter on io[k])
Rewritten as a dense bilinear contraction:
    z[b, q] = in1[b, q % 36] * in2[b, q // 36]        (q = j*36 + i, 324 rows)
    out[b]  = z[b] @ W2          with W2[j*36+i, o] = sum of cb over (i, j, o)

Per-core mapping (8-way batch data-parallel, 8192 rows/core):
  - batch tiles of 512 rows, laid out [128 partitions, 4 sub-batches]
  - PE transpose -> inT [45, 512] (features on partitions)
  - rep1 (stacked in1T) via SBUF->SBUF DMAs; rep2 (row-replicated in2T)
    via PE selection matmuls; z = rep1 * rep2 on DVE (float32r rounded)
  - out[128, 324] = sum over 3 K-chunks of z_c.T @ W2_c  (float32r matmuls,
    PSUM accumulate), copied to SBUF, DMA'd back batch-major.
"""
import sys
if '/opt/trn_rl_repo' not in sys.path:
    sys.path.insert(0, '/opt/trn_rl_repo')

import numpy as np

N_CORES = 8
B = 65536
BC = B // N_CORES          # 8192 batch rows per core
BT = 512                   # batch rows per pipeline tile
TS = BT // 128             # sub-batches per tile (4)
NT = BC // BT              # tiles per core (16)
D1, D2 = 36, 9
DF = D1 + D2               # 45
DZ = D1 * D2               # 324 z rows
DO = 324                   # out columns
NCH = 3                    # K chunks
KC = DZ // NCH             # 108 rows per chunk

_CACHE: dict = {}


def _build_program():
    import concourse.bass as bass
    import concourse.mybir as mybir
    import concourse.tile as tile
    from concourse import bacc
    from concourse.masks import make_identity

    f32 = mybir.dt.float32
    f32r = mybir.dt.float32r

    nc = bacc.Bacc("TRN2", target_bir_lowering=False, debug=False)
    in1 = nc.dram_tensor("in1", [BC, D1], f32, kind="ExternalInput").ap()
    in2 = nc.dram_tensor("in2", [BC, D2], f32, kind="ExternalInput").ap()
    w = nc.dram_tensor("w2", [NCH, KC, DO], f32r, kind="ExternalInput").ap()
    s2 = nc.dram_tensor("s2", [NCH, DF, KC], f32r, kind="ExternalInput").ap()
    out = nc.dram_tensor("out", [BC, DO], f32, kind="ExternalOutput").ap()

    with tile.TileContext(nc) as tc:
        with (
            tc.tile_pool(name="consts", bufs=1) as consts,
            tc.tile_pool(name="loads", bufs=3) as loads,
            tc.tile_pool(name="xts", bufs=3) as xts,
            tc.tile_pool(name="reps", bufs=3) as repp,
            tc.tile_pool(name="zs", bufs=2) as zp,
            tc.tile_pool(name="obs", bufs=3) as obs,
            tc.tile_pool(name="pst", bufs=2, space="PSUM") as pst,
            tc.tile_pool(name="psr", bufs=2, space="PSUM") as psr,
            tc.tile_pool(name="pso", bufs=3, space="PSUM") as pso,
        ):
            ident = consts.tile([128, 128], f32)
            make_identity(nc, ident)
            w_sb = consts.tile([KC, NCH, DO], f32r)
            nc.sync.dma_start(out=w_sb, in_=w.rearrange("c k n -> k c n"))
            s2_sb = consts.tile([DF, NCH, KC], f32r)
            nc.sync.dma_start(out=s2_sb, in_=s2.rearrange("c k n -> k c n"))

            in1r = in1.rearrange("(t p s) f -> t p s f", p=128, s=TS)
            in2r = in2.rearrange("(t p s) f -> t p s f", p=128, s=TS)
            outr = out.rearrange("(t p s) f -> t p s f", p=128, s=TS)

            for t in range(NT):
                A = loads.tile([128, TS, DF], f32)
                nc.sync.dma_start(out=A[:, :, 0:D1], in_=in1r[t])
                nc.sync.dma_start(out=A[:, :, D1:DF], in_=in2r[t])

                xT = xts.tile([DF, TS, 128], f32r)
                for s in range(TS):
                    pt = pst.tile([DF, 128], f32)
                    nc.tensor.transpose(pt, A[:, s, :], ident)
                    nc.scalar.copy(out=xT[:, s, :], in_=pt)

                rep1 = repp.tile([KC, TS, 128], f32r)
                for r in range(NCH):
                    nc.sync.dma_start(
                        out=rep1[D1 * r:D1 * (r + 1)], in_=xT[0:D1]
                    )

                zc = []
                for c in range(NCH):
                    rp = psr.tile([KC, TS * 128], f32, name="rp", tag="rp")
                    nc.tensor.matmul(
                        rp, s2_sb[:, c, :], xT[:].rearrange("k s p -> k (s p)"),
                        start=True, stop=True,
                    )
                    z = zp.tile([KC, TS, 128], f32r, name=f"z{c}")
                    nc.vector.tensor_mul(
                        z[:].rearrange("k s p -> k (s p)"),
                        rep1[:].rearrange("k s p -> k (s p)"),
                        rp,
                    )
                    zc.append(z)

                ob = obs.tile([128, TS, DO], f32)
                for s in range(TS):
                    po = pso.tile([128, DO], f32)
                    for c in range(NCH):
                        nc.tensor.matmul(
                            po, zc[c][:, s, :], w_sb[:, c, :],
                            start=(c == 0), stop=(c == NCH - 1),
                        )
                    if s % 2 == 0:
                        nc.vector.tensor_copy(out=ob[:, s, :], in_=po)
                    else:
                        nc.scalar.copy(out=ob[:, s, :], in_=po)
                nc.sync.dma_start(out=outr[t], in_=ob)

    nc.finalize()
    return nc


def _tables(in1_idx, in2_idx, out_idx, cb):
    w2 = np.zeros((DZ, DO), np.float32)
    np.add.at(
        w2,
        (in2_idx.astype(np.int64) * D1 + in1_idx.astype(np.int64),
         out_idx.astype(np.int64)),
        cb.astype(np.float32),
    )
    w2 = w2.reshape(NCH, KC, DO)
    s2 = np.zeros((NCH, DF, KC), np.float32)
    for c in range(NCH):
        for q in range(KC):
            s2[c, D1 + 3 * c + q // D1, q] = 1.0
    return w2, s2


def _get_nc():
    if "nc" not in _CACHE:
        _CACHE["nc"] = _build_program()
    return _CACHE["nc"]


def run_cores(in1, in2, w2, s2, trace=False):
    """Run the SPMD program on 8 cores; returns (out [B, DO], results obj)."""
    from concourse.bass_utils import run_bass_kernel_spmd

    nc = _get_nc()
    in_maps = []
    for c in range(N_CORES):
        in_maps.append({
            "in1": np.ascontiguousarray(in1[c * BC:(c + 1) * BC]),
            "in2": np.ascontiguousarray(in2[c * BC:(c + 1) * BC]),
            "w2": w2,
            "s2": s2,
        })
    res = run_bass_kernel_spmd(
        nc, in_maps, core_ids=list(range(N_CORES)), trace=trace
    )
    out = np.concatenate([res.results[c]["out"] for c in range(N_CORES)], axis=0)
    return out, res


def kernel(in1, in2, in1_idx, in2_idx, out_idx, cb, out_dim):
    in1 = np.asarray(in1, np.float32)
    in2 = np.asarray(in2, np.float32)
    w2, s2 = _tables(
        np.asarray(in1_idx), np.asarray(in2_idx),
        np.asarray(out_idx), np.asarray(cb),
    )
    out, _ = run_cores(in1, in2, w2, s2, trace=False)
    return out.astype(np.float32)



# revision 3
# speedup vs baseline: 1.3485x; 1.3485x over previous
"""Trainium2 Bass kernel v2 for the sparse CG tensor product.

Math: out[b, o] = sum_k cb[k] * in1[b, i1[k]] * in2[b, i2[k]]
Dense form:  z[b, q] = in1[b, q % 36] * in2[b, q // 36]  (q = j*36+i, j-major)
             out[b]  = z[b] @ W2
W2 is block-diagonal by l2-slot: rows j=0 feed only slot2=0 out cols (36),
rows j=1..3 feed slot2=1 cols (108), rows j=4..8 feed slot2=2 cols (180).
After permuting out cols slot2-major (undone on host), out splits into 4
independent matmuls with K<=128 and total streamed N = 504:
  A: z[:,   0: 36] @ W2p[  0: 36,   0: 36]
  B: z[:,  36:144] @ W2p[ 36:144,  36:144]
  C: z[:, 144:252] @ W2p[144:252, 144:324]   (PSUM accumulate with D)
  D: z[:, 252:324] @ W2p[252:324, 144:324]

Per-core pipeline (8-way batch data-parallel, 8192 rows/core, 16 tiles of 512):
  - one bf16 load [128, 4, 45] (host pre-concats in1|in2 and casts bf16)
  - 4 PE transposes -> xT [45, 4, 128] bf16 (one batched ACT copy from PSUM)
  - rep1 (in1T x3) and rep2 (in2T rows broadcast 36x) via SBUF->SBUF
    broadcast DMAs (stride-0 APs) -- no PE selection matmuls
  - z = rep1 * rep2 on DVE (bf16, 2x mode)
  - per sub-batch: 4 block matmuls (bf16, z stationary) -> PSUM [128, 324],
    copied to SBUF alternating DVE/ACT, DMA'd out batch-major fp32.
"""
import sys
if '/opt/trn_rl_repo' not in sys.path:
    sys.path.insert(0, '/opt/trn_rl_repo')

import numpy as np

N_CORES = 8
B = 65536
BC = B // N_CORES          # 8192 batch rows per core
BT = 512                   # batch rows per pipeline tile
TS = BT // 128             # sub-batches per tile (4)
NT = BC // BT              # tiles per core (16)
D1, D2 = 36, 9
DF = D1 + D2               # 45
DZ = D1 * D2               # 324
DO = 324
# chunk row ranges (j-major q): A: j0, B: j1-3, C: j4-6, D: j7-8
RA, RB, RC, RD = 36, 108, 108, 72
# permuted out col ranges: A -> [0,36), B -> [36,144), C+D -> [144,324)
CA, CB, CCD = 36, 108, 180

_CACHE: dict = {}


def _build_program(nt=NT):
    import concourse.bass as bass
    import concourse.mybir as mybir
    import concourse.tile as tile
    from concourse import bacc
    from concourse.masks import make_identity

    f32 = mybir.dt.float32
    bf16 = mybir.dt.bfloat16
    bc = nt * BT

    nc = bacc.Bacc("TRN2", target_bir_lowering=False, debug=False)
    x = nc.dram_tensor("x", [bc, DF], bf16, kind="ExternalInput").ap()
    wa = nc.dram_tensor("wa", [RA, CA], bf16, kind="ExternalInput").ap()
    wb = nc.dram_tensor("wb", [RB, CB], bf16, kind="ExternalInput").ap()
    wc = nc.dram_tensor("wc", [RC, CCD], bf16, kind="ExternalInput").ap()
    wd = nc.dram_tensor("wd", [RD, CCD], bf16, kind="ExternalInput").ap()
    s2 = nc.dram_tensor("s2", [DF, DZ], bf16, kind="ExternalInput").ap()
    out = nc.dram_tensor("out", [bc, DO], f32, kind="ExternalOutput").ap()

    with tile.TileContext(nc) as tc:
        with (
            tc.tile_pool(name="consts", bufs=1) as consts,
            tc.tile_pool(name="loads", bufs=3) as loads,
            tc.tile_pool(name="xts", bufs=3) as xts,
            tc.tile_pool(name="reps", bufs=2) as repp,
            tc.tile_pool(name="zs", bufs=2) as zp,
            tc.tile_pool(name="obs", bufs=3) as obs,
            tc.tile_pool(name="pst", bufs=1, space="PSUM") as pst,
            tc.tile_pool(name="psr", bufs=1, space="PSUM") as psr,
            tc.tile_pool(name="pso", bufs=3, space="PSUM") as pso,
        ):
            ident = consts.tile([128, 128], bf16)
            make_identity(nc, ident)
            wa_sb = consts.tile([RA, CA], bf16)
            wb_sb = consts.tile([RB, CB], bf16)
            wc_sb = consts.tile([RC, CCD], bf16)
            wd_sb = consts.tile([RD, CCD], bf16)
            s2_sb = consts.tile([DF, DZ], bf16)
            nc.sync.dma_start(out=wa_sb, in_=wa)
            nc.sync.dma_start(out=wb_sb, in_=wb)
            nc.sync.dma_start(out=wc_sb, in_=wc)
            nc.sync.dma_start(out=wd_sb, in_=wd)
            nc.sync.dma_start(out=s2_sb, in_=s2)

            xr = x.rearrange("(t p s) f -> t p s f", p=128, s=TS)
            outr = out.rearrange("(t p s) f -> t p s f", p=128, s=TS)

            for t in range(nt):
                A = loads.tile([128, TS, DF], bf16)
                nc.sync.dma_start(out=A, in_=xr[t])

                pt = pst.tile([DF, TS, 128], bf16)
                for s in range(TS):
                    nc.tensor.transpose(pt[:, s, :], A[:, s, :], ident)
                xT = xts.tile([DF, TS, 128], bf16, name="xT", tag="xT")
                nc.scalar.copy(out=xT, in_=pt)

                # rep1: in1T stacked 3x (shared by all chunks)
                rep1 = repp.tile([RB, TS, 128], bf16, name="rep1", tag="rep1")
                for r in range(3):
                    nc.sync.dma_start(
                        out=rep1[D1 * r:D1 * (r + 1)], in_=xT[0:D1]
                    )
                # rep2 via PE selection matmuls (fp32 PSUM out); s2 rows
                # 0..35 are zero so lhsT/rhs can start at partition 0
                x2f = xT[:].rearrange("k s p -> k (s p)")
                rp_a = psr.tile([RA, TS, 128], f32, name="rp_a", tag="rp_a")
                nc.tensor.matmul(
                    rp_a[:].rearrange("m s p -> m (s p)"),
                    s2_sb[:, 0:RA], x2f, start=True, stop=True)
                rp_d = psr.tile([RD, TS, 128], f32, name="rp_d", tag="rp_d")
                nc.tensor.matmul(
                    rp_d[:].rearrange("m s p -> m (s p)"),
                    s2_sb[:, RA:RA + RD], x2f, start=True, stop=True)
                rp_b = psr.tile([RB, TS, 128], f32, name="rp_b", tag="rp_b")
                nc.tensor.matmul(
                    rp_b[:].rearrange("m s p -> m (s p)"),
                    s2_sb[:, RA + RD:RA + RD + RB], x2f, start=True, stop=True)
                rp_c = psr.tile([RC, TS, 128], f32, name="rp_c", tag="rp_c")
                nc.tensor.matmul(
                    rp_c[:].rearrange("m s p -> m (s p)"),
                    s2_sb[:, RA + RD + RB:DZ], x2f, start=True, stop=True)

                za = zp.tile([RA, TS, 128], bf16, name="za", tag="za")
                nc.vector.tensor_mul(za, rep1[0:RA], rp_a)
                zd = zp.tile([RD, TS, 128], bf16, name="zd", tag="zd")
                nc.vector.tensor_mul(zd, rep1[0:RD], rp_d)
                zb = zp.tile([RB, TS, 128], bf16, name="zb", tag="zb")
                nc.vector.tensor_mul(zb, rep1, rp_b)
                zc = zp.tile([RC, TS, 128], bf16, name="zc", tag="zc")
                nc.vector.tensor_mul(zc, rep1[0:RC], rp_c)

                ob = obs.tile([128, TS, DO], f32)
                for s in range(TS):
                    po = pso.tile([128, DO], f32)
                    nc.tensor.matmul(po[:, 0:CA], za[:, s, :], wa_sb,
                                     start=True, stop=True)
                    nc.tensor.matmul(po[:, CA:CA + CB], zb[:, s, :], wb_sb,
                                     start=True, stop=True)
                    nc.tensor.matmul(po[:, CA + CB:DO], zc[:, s, :], wc_sb,
                                     start=True, stop=False)
                    nc.tensor.matmul(po[:, CA + CB:DO], zd[:, s, :], wd_sb,
                                     start=False, stop=True)
                    if s % 4 == 0:
                        nc.vector.tensor_copy(out=ob[:, s, :], in_=po)
                    else:
                        nc.scalar.copy(out=ob[:, s, :], in_=po)
                nc.sync.dma_start(out=outr[t], in_=ob)

    nc.finalize()
    return nc


def _tables(in1_idx, in2_idx, out_idx, cb):
    """Build the 4 bf16 W2 chunks + the out-column permutation."""
    import ml_dtypes
    w2 = np.zeros((DZ, DO), np.float64)
    np.add.at(
        w2,
        (in2_idx.astype(np.int64) * D1 + in1_idx.astype(np.int64),
         out_idx.astype(np.int64)),
        cb.astype(np.float64),
    )
    # out-col -> slot2 (l2-slot), derived from W2 sparsity: rows j-major,
    # j=0 -> slot2 0, j 1-3 -> 1, j 4-8 -> 2.
    j_slot2 = np.array([0] + [1] * 3 + [2] * 5)
    row_slot2 = j_slot2[np.arange(DZ) // D1]
    col_slot2 = np.full(DO, -1, np.int64)
    qs, os_ = np.nonzero(w2)
    for q, o in zip(qs, os_):
        assert col_slot2[o] in (-1, row_slot2[q])
        col_slot2[o] = row_slot2[q]
    assert (col_slot2 >= 0).all()
    perm = np.argsort(col_slot2, kind='stable')
    inv = np.argsort(perm)
    w2p = w2[:, perm]
    blocks = (
        w2p[0:36, 0:36], w2p[36:144, 36:144],
        w2p[144:252, 144:324], w2p[252:324, 144:324],
    )
    m = np.zeros_like(w2p, bool)
    m[0:36, 0:36] = m[36:144, 36:144] = m[144:324, 144:324] = True
    assert (w2p[~m] == 0).all(), "W2 leakage outside slot2 blocks"
    wa_, wb_, wc_, wd_ = (b.astype(ml_dtypes.bfloat16) for b in blocks)
    # selection matrix for rep2: cols [A(36) | D(72) | B(108) | C(108)];
    # rows 0..35 (the in1T section of xT) stay zero
    s2m = np.zeros((DF, DZ), np.float32)
    s2m[D1 + 0, 0:RA] = 1.0
    for m in range(RD):
        s2m[D1 + 7 + m // D1, RA + m] = 1.0
    for m in range(RB):
        s2m[D1 + 1 + m // D1, RA + RD + m] = 1.0
    for m in range(RC):
        s2m[D1 + 4 + m // D1, RA + RD + RB + m] = 1.0
    return (wa_, wb_, wc_, wd_), s2m.astype(ml_dtypes.bfloat16), inv


def _get_nc():
    if "nc" not in _CACHE:
        _CACHE["nc"] = _build_program()
    return _CACHE["nc"]


def run_cores(in1, in2, wchunks, s2m, trace=False):
    """Run the SPMD program on 8 cores; returns (out [B, DO] permuted, res)."""
    import ml_dtypes
    from concourse.bass_utils import run_bass_kernel_spmd

    nc = _get_nc()
    xfull = np.concatenate(
        [np.asarray(in1, np.float32), np.asarray(in2, np.float32)], axis=1
    ).astype(ml_dtypes.bfloat16)
    wa_, wb_, wc_, wd_ = wchunks
    in_maps = []
    for c in range(N_CORES):
        in_maps.append({
            "x": np.ascontiguousarray(xfull[c * BC:(c + 1) * BC]),
            "wa": wa_, "wb": wb_, "wc": wc_, "wd": wd_, "s2": s2m,
        })
    res = run_bass_kernel_spmd(
        nc, in_maps, core_ids=list(range(N_CORES)), trace=trace
    )
    outp = np.concatenate([res.results[c]["out"] for c in range(N_CORES)], axis=0)
    return outp, res


def kernel(in1, in2, in1_idx, in2_idx, out_idx, cb, out_dim):
    wchunks, s2m, inv = _tables(
        np.asarray(in1_idx), np.asarray(in2_idx),
        np.asarray(out_idx), np.asarray(cb),
    )
    outp, _ = run_cores(in1, in2, wchunks, s2m, trace=False)
    return np.ascontiguousarray(outp[:, inv]).astype(np.float32)
